# revision 1
# baseline (speedup 1.0000x reference)
"""Trainium2 Bass kernel for the GCNEncoder problem.

Strategy:
  - Pure data parallelism: batch 65536 split as 8192 per core across 8 cores.
  - Host-side folding (weights are tiny):
      C1 = kron(A, W1)            -- layer-1 graph-mix + lift fused: (1088, 51)
      C2 = kron(A, W2)            -- layer-2 fused, block-sparse over joint pairs
      D  = Wp1 @ kron(A, W3)      -- layer-3 + pool-layer-1 collapse: (64, 1088)
  - x is transposed host-side to (51, B) so activations stay
    feature-on-partition on device with perfectly contiguous DMA.
  - Device pipeline per 512-sample tile:
      stage A: 9 matmuls (K=51)               -> H1 (128, 9*512) pair layout
      stage B: 18 pair-block matmuls (K<=128) -> H2 (128, 9*512)
      stage C: 9 accumulating matmuls         -> z4 (64, 512)
      stage D: 4 matmuls with z4 as lhsT      -> out (samples, 256) directly
    Bias+ReLU fused into PSUM evacuation, split across ScalarE/VectorE.
    Joints are paired as siblings so stage B needs only 18 blocks (vs 24).
  - float32r matmuls (single-pass fp32, ~1e-4 matmul rel err, 4x faster
    than the fp32 two-pass path).
"""

import os
import sys

for _p in ("/opt/trn_rl_repo", "/root/.axon_site/_ro/trn_rl_repo"):
    if os.path.isdir(_p) and _p not in sys.path:
        sys.path.insert(0, _p)

import numpy as np

from concourse import bacc, mybir, tile
from concourse.bass_utils import run_bass_kernel_spmd

NJ = 17            # joints
DIN = 3            # input dims per joint
H = 64             # hidden per joint
DOUT = 256
NCORES = 8
B_TOTAL = 65536
BC = B_TOTAL // NCORES          # 8192 per core
TILE_N = 512                    # samples per device tile
CHUNKS_PER_TILE = TILE_N // 128  # 4
NTILES = BC // TILE_N           # 16
NCHUNKS = BC // 128             # 64

F32 = mybir.dt.float32
F32R = mybir.dt.float32r
BF16 = mybir.dt.bfloat16
B_BF16 = bool(int(os.environ.get("KERNEL_B_BF16", "0")))
BDT = BF16 if B_BF16 else F32R

# Joint pairing chosen to minimize nonzero pair-blocks of kron(A, W2):
# siblings (nodes sharing a neighbor, never adjacent) share their neighbor
# sets, so the 16 tree edges collapse into 9 unordered pair-pairs (18
# ordered blocks) and no diagonal blocks. Natural pairing gives 24.
PAIRS = [(1, 3), (4, 6), (8, 10), (11, 13), (14, 16), (7, 9), (0, 2), (5, 12), (15,)]
PERM = [j for pq in PAIRS for j in pq]          # joint order, row-block major

LAST_RESULTS = None  # stash of BassKernelResults for test harness introspection


def _build_constants(A, W1, b1, W2, b2, W3, b3, Wp1, bp1, Wp2, bp2):
    """Host-side folding. All fp32 numpy."""
    A = np.asarray(A, np.float32)
    C1 = np.kron(A, np.asarray(W1, np.float32))            # (1088, 51)
    C2 = np.kron(A, np.asarray(W2, np.float32))            # (1088, 1088)
    C3 = np.kron(A, np.asarray(W3, np.float32))            # (1088, 1088)
    D = np.asarray(Wp1, np.float32) @ C3                   # (64, 1088)
    bp1p = (np.asarray(Wp1, np.float32) @ np.tile(np.asarray(b3, np.float32), NJ)
            + np.asarray(bp1, np.float32))                 # (64,)

    # permute joint-major rows into PAIRS order
    perm_rows = np.concatenate([np.arange(j * H, (j + 1) * H) for j in PERM])
    C1 = C1[perm_rows]
    C2 = C2[perm_rows][:, perm_rows]
    D = D[:, perm_rows]

    # G1: lhsT chunks of C1, concatenated along free dim. chunk q is (51, Mq)
    g1 = C1.T.copy()                                       # (51, 1088)

    # G2: nonzero pair blocks of C2, transposed, concatenated along free dim
    row_off = [128 * q for q in range(9)]
    blocks = []   # (q, p, coloff, K, M)
    cols = []
    coloff = 0
    for q, pq in enumerate(PAIRS):
        Mq = H * len(pq)
        for p, pp in enumerate(PAIRS):
            Kp = H * len(pp)
            blk = C2[row_off[q]:row_off[q] + Mq, row_off[p]:row_off[p] + Kp]
            if np.abs(blk).max() == 0.0:
                continue
            t = np.zeros((128, Mq), np.float32)
            t[:Kp, :] = blk.T
            blocks.append((q, p, coloff, Kp, Mq))
            cols.append(t)
            coloff += Mq
    g2 = np.concatenate(cols, axis=1)                      # (128, ~3008)

    # G3: D.T chunks (Kp, 64) at columns 64*p
    g3 = np.zeros((128, 9 * H), np.float32)
    for p, pp in enumerate(PAIRS):
        Kp = H * len(pp)
        g3[:Kp, H * p:H * (p + 1)] = D[:, row_off[p]:row_off[p] + Kp].T

    consts = {
        "g1": g1,
        "g2": g2,
        "g3": g3,
        "wp2t": np.asarray(Wp2, np.float32).T.copy(),      # (64, 256)
        "b1p": np.tile(np.asarray(b1, np.float32), 2).reshape(128, 1).copy(),
        "b2p": np.tile(np.asarray(b2, np.float32), 2).reshape(128, 1).copy(),
        "bp1p": bp1p.reshape(64, 1).copy(),
        "bp2b": np.tile(np.asarray(bp2, np.float32), (128, 2)).copy(),  # (128, 512)
    }
    return consts, blocks


def _build_program(blocks, reps=1):
    probe = os.environ.get("KERNEL_PROBE", "none")
    nc = bacc.Bacc(None)

    x_d = nc.declare_dram_parameter("x", [NJ * DIN, BC], F32R, isOutput=False)
    g1_d = nc.declare_dram_parameter("g1", [NJ * DIN, NJ * H], F32R, isOutput=False)
    g2_cols = max(b[2] + b[4] for b in blocks)
    g2_d = nc.declare_dram_parameter("g2", [128, g2_cols], BDT, isOutput=False)
    g3_d = nc.declare_dram_parameter("g3", [128, 9 * H], F32R, isOutput=False)
    wp2t_d = nc.declare_dram_parameter("wp2t", [H, DOUT], F32R, isOutput=False)
    b1p_d = nc.declare_dram_parameter("b1p", [128, 1], F32, isOutput=False)
    b2p_d = nc.declare_dram_parameter("b2p", [128, 1], F32, isOutput=False)
    bp1p_d = nc.declare_dram_parameter("bp1p", [H, 1], F32, isOutput=False)
    bp2b_d = nc.declare_dram_parameter("bp2b", [128, 512], F32, isOutput=False)
    out_d = nc.declare_dram_parameter("out", [BC, DOUT], F32, isOutput=True)

    out_r = out_d.rearrange("(c p) f -> p c f", p=128)      # (128, 64, 256)

    AF = mybir.ActivationFunctionType
    ALU = mybir.AluOpType

    # blocks grouped by output pair q
    blocks_by_q = [[b for b in blocks if b[0] == q] for q in range(9)]

    with tile.TileContext(nc) as tc:
        with (
            tc.tile_pool(name="const", bufs=1) as cp,
            tc.tile_pool(name="h1", bufs=2) as h1p,
            tc.tile_pool(name="h2", bufs=2) as h2p,
            tc.tile_pool(name="z4", bufs=2) as z4p,
            tc.tile_pool(name="ot", bufs=3) as otp,
            tc.tile_pool(name="psa", bufs=3, space="PSUM") as psa,
            tc.tile_pool(name="psb", bufs=3, space="PSUM") as psb,
            tc.tile_pool(name="psc", bufs=1, space="PSUM") as psc,
            tc.tile_pool(name="psd", bufs=1, space="PSUM") as psd,
        ):
            x_all = cp.tile([NJ * DIN, BC], F32R)
            g1_sb = cp.tile([NJ * DIN, NJ * H], F32R)
            g2_sb = cp.tile([128, g2_cols], BDT)
            g3_sb = cp.tile([128, 9 * H], F32R)
            wp2t_sb = cp.tile([H, DOUT], F32R)
            b1p_sb = cp.tile([128, 1], F32)
            b2p_sb = cp.tile([128, 1], F32)
            bp1p_sb = cp.tile([H, 1], F32)
            bp2b_sb = cp.tile([128, 512], F32)

            nc.sync.dma_start(x_all[:], x_d[:])
            nc.sync.dma_start(g1_sb[:], g1_d[:])
            nc.sync.dma_start(g2_sb[:], g2_d[:])
            nc.sync.dma_start(g3_sb[:], g3_d[:])
            nc.sync.dma_start(wp2t_sb[:], wp2t_d[:])
            nc.sync.dma_start(b1p_sb[:], b1p_d[:])
            nc.sync.dma_start(b2p_sb[:], b2p_d[:])
            nc.sync.dma_start(bp1p_sb[:], bp1p_d[:])
            nc.sync.dma_start(bp2b_sb[:], bp2b_d[:])
            if int(os.environ.get("KERNEL_SALT", "0")):
                salt_sb = cp.tile([128, 1 + int(os.environ["KERNEL_SALT"])], F32)
                nc.gpsimd.memset(salt_sb[:], 0.0)

            def evac_relu_act(dst, src, bias):
                nc.scalar.activation(dst, src, AF.Relu, bias=bias)

            def evac_relu_dve(dst, src, bias):
                nc.vector.tensor_scalar(
                    out=dst, in0=src, scalar1=bias, scalar2=0.0,
                    op0=ALU.add, op1=ALU.max,
                )

            def tile_body(t):
                xt_sb = x_all[:, TILE_N * t:TILE_N * (t + 1)]

                # ---- stage A: H1 = relu(C1 @ x + b1), pair layout
                h1_sb = h1p.tile([128, 9 * TILE_N], BDT)
                for q, pq in enumerate(PAIRS):
                    Mq = H * len(pq)
                    ps_a = psa.tile([128, TILE_N], F32)
                    if probe != "noA":
                        nc.tensor.matmul(
                            ps_a[0:Mq, :],
                            g1_sb[:, 128 * q:128 * q + Mq],
                            xt_sb[:],
                            start=True, stop=True,
                        )
                    dst = h1_sb[0:Mq, TILE_N * q:TILE_N * (q + 1)]
                    if q % 2 == 0:
                        evac_relu_act(dst, ps_a[0:Mq, :], b1p_sb[0:Mq, :])
                    else:
                        evac_relu_dve(dst, ps_a[0:Mq, :], b1p_sb[0:Mq, :])

                # ---- stage B: H2 = relu(C2 @ H1 + b2), pair-block sparse
                h2_sb = h2p.tile([128, 9 * TILE_N], F32R)
                for q, pq in enumerate(PAIRS):
                    Mq = H * len(pq)
                    ps_b = psb.tile([128, TILE_N], F32)
                    bq = blocks_by_q[q]
                    for bi, (_, p, coloff, Kp, Mq2) in enumerate(bq):
                        if probe == "noB":
                            continue
                        if probe == "sameW":
                            coloff = 0
                            Kp = 128
                            Mq_w = 128 if Mq == 128 else Mq
                        nc.tensor.matmul(
                            ps_b[0:Mq, :],
                            g2_sb[0:Kp, coloff:coloff + Mq],
                            h1_sb[0:Kp, TILE_N * p:TILE_N * (p + 1)],
                            start=(bi == 0), stop=(bi == len(bq) - 1),
                        )
                    dst = h2_sb[0:Mq, TILE_N * q:TILE_N * (q + 1)]
                    if q % 2 == 0:
                        evac_relu_act(dst, ps_b[0:Mq, :], b2p_sb[0:Mq, :])
                    else:
                        evac_relu_dve(dst, ps_b[0:Mq, :], b2p_sb[0:Mq, :])

                # ---- stage C: z4 = relu(D @ H2 + bp1'), single accumulator
                ps_c = psc.tile([H, TILE_N], F32)
                for p, pp in enumerate(PAIRS):
                    if probe == "noC":
                        continue
                    Kp = H * len(pp)
                    nc.tensor.matmul(
                        ps_c[:],
                        g3_sb[0:Kp, H * p:H * (p + 1)],
                        h2_sb[0:Kp, TILE_N * p:TILE_N * (p + 1)],
                        start=(p == 0), stop=(p == 8),
                    )
                z4_sb = z4p.tile([H, TILE_N], F32R)
                evac_relu_act(z4_sb[:], ps_c[:], bp1p_sb[:])

                # ---- stage D: out = z4.T @ Wp2T + bp2 (samples on partitions)
                ot_sb = otp.tile([128, CHUNKS_PER_TILE * DOUT], F32)
                for half in range(2):
                    ps_d = psd.tile([128, 512], F32)
                    for k in range(2):
                        kk = 2 * half + k
                        nc.tensor.matmul(
                            ps_d[:, DOUT * k:DOUT * (k + 1)],
                            z4_sb[:, 128 * kk:128 * (kk + 1)],
                            wp2t_sb[:],
                            start=True, stop=True,
                        )
                    nc.vector.tensor_tensor(
                        out=ot_sb[:, 512 * half:512 * (half + 1)],
                        in0=ps_d[:],
                        in1=bp2b_sb[:],
                        op=ALU.add,
                    )
                nc.sync.dma_start(
                    out_r[:, CHUNKS_PER_TILE * t:CHUNKS_PER_TILE * (t + 1), :],
                    ot_sb[:],
                )

            if reps == 1:
                for t in range(NTILES):
                    tile_body(t)
            else:
                with tc.For_i(0, reps, 1):
                    for t in range(NTILES):
                        tile_body(t)

    nc.compile()
    return nc


_CACHE = {}


def kernel(**inputs):
    global LAST_RESULTS
    x = np.ascontiguousarray(np.asarray(inputs["x"], np.float32))
    consts, blocks = _build_constants(
        inputs["A"], inputs["W1"], inputs["b1"], inputs["W2"], inputs["b2"],
        inputs["W3"], inputs["b3"], inputs["Wp1"], inputs["bp1"],
        inputs["Wp2"], inputs["bp2"],
    )

    reps = int(os.environ.get("BENCH_REPS", "1"))
    key = (reps, B_BF16) + tuple(b[:3] for b in blocks)
    if key not in _CACHE:
        _CACHE[key] = _build_program(blocks, reps=reps)
    nc = _CACHE[key]

    if B_BF16:
        import ml_dtypes
        consts["g2"] = consts["g2"].astype(ml_dtypes.bfloat16)
    xf = x.reshape(B_TOTAL, NJ * DIN)
    in_maps = []
    for c in range(NCORES):
        m = dict(consts)
        m["x"] = np.ascontiguousarray(xf[c * BC:(c + 1) * BC].T)
        in_maps.append(m)

    res = run_bass_kernel_spmd(nc, in_maps, list(range(NCORES)))
    LAST_RESULTS = res
    out = np.concatenate([res.results[c]["out"] for c in range(NCORES)], axis=0)
    return out.astype(np.float32, copy=False)

